# revision 5
# baseline (speedup 1.0000x reference)
"""Bahdanau additive attention kernel for 8 Trainium2 NeuronCores.

Math (per batch element b):
    pq = query[b] @ Wq.T                       [Q, NU]
    pk = keys[b]  @ Wk.T                       [K, NU]
    v  = linear_att / ||linear_att|| * normalize_scalar
    scores[q,k] = sum_u tanh(pq[q,u] + pk[k,u] + bias[u]) * v[u]
    scores_normalized = softmax(scores, -1)
    context = scores @ keys[b]                 (un-normalized scores, faithful)

Sharding: data parallel over batch, B == 8 == n_cores, no collectives.

Per-core pipeline:
    PE   : pqT[u,q], pkT[u,k] projections (fp32 matmuls)
    DVE  : S[u, (q,k)-chunk] = pkT + pq[q]   (tensor_scalar add, 2x mode)
    ACT  : T = tanh(S) in large-free-dim instructions, output fp16
    PE   : scoresT[k,q] = sum_u T[u,k] * v[u]  (fp16 matvec, PSUM accum)
    PE/DVE/ACT: transpose + softmax + context matmul
"""

import sys

for _p in ("/opt/trn_rl_repo",):
    if _p not in sys.path:
        sys.path.insert(0, _p)

import numpy as np

B, Q, K, D, NU = 8, 64, 512, 512, 512
UT = NU // 128  # u tiles
KT = K // 128   # k tiles
DT = D // 128   # d tiles
QB = 8          # q's per hot-loop chunk
N_CORES = 8

_CACHE = {}


def _build():
    from contextlib import ExitStack
    from concourse import bacc, tile, mybir
    import concourse.bass as bass
    from concourse.masks import make_identity

    f32 = mybir.dt.float32
    f16 = mybir.dt.float16

    nc = bacc.Bacc("TRN2", target_bir_lowering=False, debug=False,
                   num_devices=N_CORES)

    qT_ap = nc.dram_tensor("qT", [D, Q], f32, kind="ExternalInput").ap()
    keys_ap = nc.dram_tensor("keys", [K, D], f32, kind="ExternalInput").ap()
    keysT_ap = nc.dram_tensor("keysT", [D, K], f32, kind="ExternalInput").ap()
    wqT_ap = nc.dram_tensor("wqT", [D, NU], f32, kind="ExternalInput").ap()
    wkT_ap = nc.dram_tensor("wkT", [D, NU], f32, kind="ExternalInput").ap()
    v16_ap = nc.dram_tensor("v16", [128, UT], f16, kind="ExternalInput").ap()
    biasb_ap = nc.dram_tensor("biasb", [128, UT], f32, kind="ExternalInput").ap()
    ctx_out_ap = nc.dram_tensor("ctx_out", [Q, D], f32, kind="ExternalOutput").ap()
    sn_out_ap = nc.dram_tensor("sn_out", [Q, K], f32, kind="ExternalOutput").ap()

    Tanh = mybir.ActivationFunctionType.Tanh
    Exp = mybir.ActivationFunctionType.Exp

    with tile.TileContext(nc) as tc:
        with ExitStack() as ctx:
            singles = ctx.enter_context(tc.tile_pool(name="singles", bufs=1))
            s_pool = ctx.enter_context(tc.tile_pool(name="s", bufs=3))
            t_pool = ctx.enter_context(tc.tile_pool(name="t", bufs=8))
            ps_proj = ctx.enter_context(
                tc.tile_pool(name="ps_proj", bufs=2, space="PSUM"))
            ps_sc = ctx.enter_context(
                tc.tile_pool(name="ps_sc", bufs=1, space="PSUM"))
            ps_tail = ctx.enter_context(
                tc.tile_pool(name="ps_tail", bufs=1, space="PSUM"))

            # ---- input tiles -------------------------------------------------
            sb_qT = singles.tile([128, DT, Q], f32)
            sb_keys = singles.tile([128, KT, D], f32)
            sb_keysT = singles.tile([128, DT, K], f32)
            sb_wqT = singles.tile([128, DT, NU], f32)
            sb_wkT = singles.tile([128, DT, NU], f32)
            sb_v16 = singles.tile([128, UT], f16)
            sb_biasb = singles.tile([128, UT], f32)
            for t in range(DT):
                nc.sync.dma_start(out=sb_qT[:, t, :], in_=qT_ap[t * 128:(t + 1) * 128, :])
                nc.sync.dma_start(out=sb_keysT[:, t, :], in_=keysT_ap[t * 128:(t + 1) * 128, :])
                nc.sync.dma_start(out=sb_wqT[:, t, :], in_=wqT_ap[t * 128:(t + 1) * 128, :])
                nc.sync.dma_start(out=sb_wkT[:, t, :], in_=wkT_ap[t * 128:(t + 1) * 128, :])
            for t in range(KT):
                nc.sync.dma_start(out=sb_keys[:, t, :], in_=keys_ap[t * 128:(t + 1) * 128, :])
            nc.sync.dma_start(out=sb_v16[:, :], in_=v16_ap[:, :])
            nc.sync.dma_start(out=sb_biasb[:, :], in_=biasb_ap[:, :])

            identity = singles.tile([128, 128], f32)
            make_identity(nc, identity[:, :])

            # ---- projections: pqT[u,q] (+bias), pkT[u,k] ---------------------
            sb_pqT = singles.tile([128, UT, Q], f32)
            sb_pkT = singles.tile([128, UT, K], f32)
            for ut in range(UT):
                pq_ps = ps_proj.tile([128, Q], f32, tag="pq")
                for dt in range(DT):
                    nc.tensor.matmul(
                        out=pq_ps[:, :],
                        lhsT=sb_wqT[:, dt, ut * 128:(ut + 1) * 128],
                        rhs=sb_qT[:, dt, :],
                        start=(dt == 0), stop=(dt == DT - 1))
                # fold normalize_bias while copying out of PSUM
                nc.vector.tensor_scalar_add(
                    out=sb_pqT[:, ut, :], in0=pq_ps[:, :],
                    scalar1=sb_biasb[:, ut:ut + 1])
                pk_ps = ps_proj.tile([128, K], f32, tag="pk")
                for dt in range(DT):
                    nc.tensor.matmul(
                        out=pk_ps[:, :],
                        lhsT=sb_wkT[:, dt, ut * 128:(ut + 1) * 128],
                        rhs=sb_keysT[:, dt, :],
                        start=(dt == 0), stop=(dt == DT - 1))
                nc.vector.tensor_copy(sb_pkT[:, ut, :], pk_ps[:, :])

            # ---- hot loop: S = pkT + pq[q]; T = tanh(S); scoresT += T.v -----
            psum_scT = ps_sc.tile([128, KT, Q], f32)  # [k-tile part, kt, q]
            for qb in range(Q // QB):
                Ts = []
                for ut in range(UT):
                    S = s_pool.tile([128, QB, K], f32)
                    for j in range(QB):
                        q = qb * QB + j
                        nc.vector.tensor_scalar_add(
                            out=S[:, j, :], in0=sb_pkT[:, ut, :],
                            scalar1=sb_pqT[:, ut, q:q + 1])
                    T = t_pool.tile([128, QB, K], f16)
                    nc.scalar.activation(T[:, :, :], S[:, :, :], Tanh)
                    Ts.append(T)
                for j in range(QB):
                    q = qb * QB + j
                    for kt in range(KT):
                        for ut in range(UT):
                            nc.tensor.matmul(
                                out=psum_scT[:, kt, q:q + 1],
                                lhsT=Ts[ut][:, j, kt * 128:(kt + 1) * 128],
                                rhs=sb_v16[:, ut:ut + 1],
                                start=(ut == 0), stop=(ut == UT - 1))

            # ---- scoresT -> sbuf; transpose to scores[q,k] ------------------
            scT_sb = singles.tile([128, KT, Q], f32)
            nc.vector.tensor_copy(scT_sb[:, :, :], psum_scT[:, :, :])

            psum_sc = ps_tail.tile([64, K], f32, tag="sc")
            for kt in range(KT):
                nc.tensor.transpose(
                    out=psum_sc[:, kt * 128:(kt + 1) * 128],
                    in_=scT_sb[:, kt, :], identity=identity[:, :])

            # ---- softmax over k ---------------------------------------------
            negmax = singles.tile([64, 1], f32)
            nc.vector.tensor_reduce(
                out=negmax[:, :], in_=psum_sc[:, :],
                axis=mybir.AxisListType.X, op=mybir.AluOpType.max, negate=True)
            Etile = singles.tile([64, K], f32)
            nc.scalar.activation(Etile[:, :], psum_sc[:, :], Exp,
                                 bias=negmax[:, :])
            ssum = singles.tile([64, 1], f32)
            nc.vector.tensor_reduce(
                out=ssum[:, :], in_=Etile[:, :],
                axis=mybir.AxisListType.X, op=mybir.AluOpType.add)
            rinv = singles.tile([64, 1], f32)
            nc.vector.reciprocal(rinv[:, :], ssum[:, :])
            SN = singles.tile([64, K], f32)
            nc.vector.tensor_scalar_mul(out=SN[:, :], in0=Etile[:, :],
                                        scalar1=rinv[:, :])
            nc.sync.dma_start(out=sn_out_ap[:, :], in_=SN[:, :])

            # ---- context = scores @ keys (un-normalized scores) -------------
            psum_ctx = ps_tail.tile([64, D], f32, tag="ctx")
            for kt in range(KT):
                nc.tensor.matmul(
                    out=psum_ctx[:, :],
                    lhsT=scT_sb[:, kt, :],
                    rhs=sb_keys[:, kt, :],
                    start=(kt == 0), stop=(kt == KT - 1))
            ctx_sb = singles.tile([64, D], f32)
            nc.vector.tensor_copy(ctx_sb[:, :], psum_ctx[:, :])
            nc.sync.dma_start(out=ctx_out_ap[:, :], in_=ctx_sb[:, :])

    nc.compile()
    return nc


def _get_nc():
    if "nc" not in _CACHE:
        _CACHE["nc"] = _build()
    return _CACHE["nc"]


def _prep_inputs(query, keys, Wq, Wk, linear_att, normalize_scalar,
                 normalize_bias):
    query = np.asarray(query, dtype=np.float32)
    keys = np.asarray(keys, dtype=np.float32)
    Wq = np.asarray(Wq, dtype=np.float32)
    Wk = np.asarray(Wk, dtype=np.float32)
    linear_att = np.asarray(linear_att, dtype=np.float32)
    normalize_scalar = np.asarray(normalize_scalar, dtype=np.float32)
    normalize_bias = np.asarray(normalize_bias, dtype=np.float32)

    v = (linear_att / np.linalg.norm(linear_att)) * normalize_scalar[0]
    v16 = np.ascontiguousarray(v.reshape(UT, 128).T).astype(np.float16)
    biasb = np.ascontiguousarray(normalize_bias.reshape(UT, 128).T)
    wqT = np.ascontiguousarray(Wq.T)
    wkT = np.ascontiguousarray(Wk.T)

    in_maps = []
    for b in range(B):
        in_maps.append({
            "qT": np.ascontiguousarray(query[b].T),
            "keys": np.ascontiguousarray(keys[b]),
            "keysT": np.ascontiguousarray(keys[b].T),
            "wqT": wqT,
            "wkT": wkT,
            "v16": v16,
            "biasb": biasb,
        })
    return in_maps


def kernel(query, keys, Wq, Wk, linear_att, normalize_scalar, normalize_bias):
    from concourse.bass_utils import run_bass_kernel_spmd

    nc = _get_nc()
    in_maps = _prep_inputs(query, keys, Wq, Wk, linear_att, normalize_scalar,
                           normalize_bias)
    res = run_bass_kernel_spmd(nc, in_maps, core_ids=list(range(N_CORES)))
    context = np.stack([res.results[b]["ctx_out"] for b in range(B)])
    scores_normalized = np.stack([res.results[b]["sn_out"] for b in range(B)])
    return context.astype(np.float32), scores_normalized.astype(np.float32)
